# revision 3
# baseline (speedup 1.0000x reference)
"""NeuralFactorizationMachine Trainium2 kernel v2 (8 NeuronCores, SPMD).

Reference computation (B=1024, N=16384, D=512, O=4096):
    sum_emb = sae @ emb                      (B, D)
    sum_sq  = (sae*sae) @ (emb*emb)          (B, D)
    inter   = 0.5*(sum_emb^2 - sum_sq)       (B, D)
    h       = relu(inter @ mlp1_w.T + b1)    (B, D)
    out     = h @ mlp2_w.T + b2 + sae @ linear_w.T + lb   (B, O)

Sharding (8 cores), v2 = batch-sharded FM (no big AllReduce):
  - FM GEMMs (sum_emb/sum_sq) are DATA-parallel over batch: core c owns
    batch rows c*128..(c+1)*128 and contracts over the FULL N=16384 with
    the full emb table (streamed).  This kills v1's 4 MiB fp32 AllReduce
    (~200 us of critical path) entirely.
  - mlp1 is batch-sharded too (h shard [128b, D] per core), then a tiny
    AllGather (128 KiB bf16 per rank) replicates h on every core; the
    gathered [B, D] block is loaded back via hardware DMA-transpose to
    give the [d, b] lhsT tiles mlp2 needs.
  - linear / mlp2 are tensor-parallel over O (each core owns 512 output
    cols); mlp2's 4 matmuls are accumulated INTO the linear GEMM's PSUM
    banks before stop, so the tail is just bias + copy + store.
  - Final output is a host-side concat over O.

All large inputs are host-swizzled to [128, ...] partition-major flat
layouts so every big DMA moves contiguous multi-KiB runs per partition.
DMA queue assignment: sync = saeb + sa stream + out; scalar = consts +
emb even chunks + lw stream + h transpose-loads; gpsimd(SWDGE) = saebsq
+ emb odd chunks + h_mine store + AllGather trigger.  GEMM inputs bf16,
fp32 PSUM accumulation.
"""

import numpy as np
import ml_dtypes

import concourse.bass as bass
import concourse.mybir as mybir
import concourse.tile as tile
from concourse import bacc
from concourse.bass_utils import run_bass_kernel_spmd

B, N, D, O = 1024, 16384, 512, 4096
C = 8                # cores
BS = B // C          # 128 batch rows per core (FM batch shard)
OS = O // C          # 512 output cols per core
BF16 = mybir.dt.bfloat16
F32 = mybir.dt.float32

KT = N // 128        # 128 n-ktiles total
EC = 4               # emb ktiles per streamed chunk
NCH = KT // EC       # 32 emb chunks
DT = D // 128        # 4 d-tiles
MT = B // 128        # 8 m-tiles (batch) for p2
NB = 16              # n-blocks for the linear GEMM stream
NTB = KT // NB       # 8 n-tiles per block
SBP = 8              # saeb load split


def _build(repeat=1, phases=("fm", "ag", "p2", "tail")):
    nc = bacc.Bacc(
        "TRN2",
        target_bir_lowering=False,
        debug=False,
        enable_asserts=False,
        num_devices=C,
    )

    # host-swizzled flat [128, ...] layouts (k-tile-major within each row)
    saeTd = nc.dram_tensor("saeTd", [128, KT * B], BF16, kind="ExternalInput").ap()
    saebd = nc.dram_tensor("saebd", [128, KT * BS], BF16, kind="ExternalInput").ap()
    embd = nc.dram_tensor("embd", [128, KT * D], BF16, kind="ExternalInput").ap()
    linwd = nc.dram_tensor("linwd", [128, KT * OS], BF16, kind="ExternalInput").ap()
    mlp1wT = nc.dram_tensor("mlp1wT", [D, D], BF16, kind="ExternalInput").ap()
    mlp1brow = nc.dram_tensor("mlp1brow", [1, D], BF16, kind="ExternalInput").ap()
    mlp2wT = nc.dram_tensor("mlp2wT", [D, OS], BF16, kind="ExternalInput").ap()
    biasrow = nc.dram_tensor("biasrow", [1, OS], BF16, kind="ExternalInput").ap()
    ident = nc.dram_tensor("ident", [128, 128], BF16, kind="ExternalInput").ap()
    out = nc.dram_tensor("out", [B, OS], F32, kind="ExternalOutput").ap()

    with tile.TileContext(nc) as tc:
      for rep in range(repeat):
        with (
            tc.tile_pool(name=f"dram{rep}", bufs=1, space="DRAM") as dram,
            tc.tile_pool(name=f"const{rep}", bufs=1) as cst,
            tc.tile_pool(name=f"p2sa{rep}", bufs=2) as p2sa,
            tc.tile_pool(name=f"p2lw{rep}", bufs=2) as p2lw,
            tc.tile_pool(name=f"p2st{rep}", bufs=4) as p2st,
            tc.tile_pool(name=f"fme{rep}", bufs=4) as fme,
        ):
            # prefetch the first emb chunks ahead of the consts so the FM
            # matmuls can start as early as possible
            ew_pre = []
            if "fm" in phases:
                for ch in range(2):
                    csl = slice(ch * EC * D, (ch + 1) * EC * D)
                    ew = fme.tile([128, EC * D], BF16, tag="ew", name="ew")
                    eng = nc.scalar if ch % 2 == 0 else nc.gpsimd
                    eng.dma_start(ew[:], embd[:, csl])
                    ew_pre.append(ew)

            # const tiles; DMAs are emitted after the FM stream so the
            # scalar queue serves the emb chunks first (consts are only
            # needed from ~60us on)
            w1 = cst.tile([128, DT * D], BF16, tag="w1", name="w1")
            b1r = cst.tile([1, D], BF16, tag="b1r", name="b1r")
            w2 = cst.tile([128, DT * OS], BF16, tag="w2", name="w2")
            br = cst.tile([1, OS], BF16, tag="br", name="br")
            idt = cst.tile([128, 128], BF16, tag="idt", name="idt")
            ones = cst.tile([1, 128], BF16, tag="ones", name="ones")
            nc.vector.memset(ones[:], 1.0)

            def load_consts():
                nc.scalar.dma_start(
                    w1[:], mlp1wT.rearrange("(k p) d -> p k d", p=128))
                nc.scalar.dma_start(b1r[:], mlp1brow[:, :])
                nc.scalar.dma_start(
                    w2[:], mlp2wT.rearrange("(k p) o -> p k o", p=128))
                nc.scalar.dma_start(br[:], biasrow[:, :])
                nc.scalar.dma_start(idt[:], ident[:, :])

            # h gathered from all cores: [B, D] bf16 (AllGather output)
            h_all = dram.tile([C * BS, D], BF16, tag="h_all",
                              name=f"h_all{rep}", addr_space="Shared")
            h_mine = dram.tile([BS, D], BF16, tag="h_mine",
                               name=f"h_mine{rep}")

            # hT tiles for mlp2 (filled by DMA-transpose from h_all):
            # htall[p, kd*B + b] = h_all[b, kd*128 + p]
            htall = cst.tile([128, DT * B], BF16, tag="htall", name="htall")

            # ---------------- Phase FM: batch-sharded FM GEMMs ------------
            if "fm" in phases:
              with (
                tc.tile_pool(name=f"fmw{rep}", bufs=1) as fmw,
                tc.tile_pool(name=f"fmps{rep}", bufs=1, space="PSUM") as fmps,
                tc.tile_pool(name=f"fmtp{rep}", bufs=2, space="PSUM") as fmtp,
                tc.tile_pool(name=f"fmhp{rep}", bufs=1, space="PSUM") as fmhp,
                tc.tile_pool(name=f"fmst{rep}", bufs=2) as fmst,
              ):
                # stationary operands: core's saeT column block, split into
                # SBP sub-DMAs so the first matmuls start early; squares
                # computed on DVE per sub-part (saves 4.2 MiB of DMA in the
                # critical early window).
                saeb = fmw.tile([128, KT * BS], BF16, tag="saeb", name="saeb")
                saebsq = fmw.tile([128, KT * BS], BF16, tag="saebsq",
                                  name="saebsq")
                part = KT * BS // SBP
                for s in range(SBP):
                    sl = slice(s * part, (s + 1) * part)
                    nc.sync.dma_start(saeb[:, sl], saebd[:, sl])
                    nc.vector.tensor_mul(saebsq[:, sl], saeb[:, sl],
                                         saeb[:, sl])

                pse = fmps.tile([128, D], F32, tag="pse", name="pse")
                psq = fmps.tile([128, D], F32, tag="psq", name="psq")

                for ch in range(NCH):
                    csl = slice(ch * EC * D, (ch + 1) * EC * D)
                    if ch < len(ew_pre):
                        ew = ew_pre[ch]
                    else:
                        ew = fme.tile([128, EC * D], BF16, tag="ew", name="ew")
                        # alternate queues so the emb stream isn't
                        # serialized behind a single DMA ring
                        eng = nc.scalar if ch % 2 == 0 else nc.gpsimd
                        eng.dma_start(ew[:], embd[:, csl])
                    esq = fme.tile([128, EC * D], BF16, tag="esq", name="esq")
                    nc.vector.tensor_mul(esq[:], ew[:], ew[:])
                    for kt in range(EC):
                        k = ch * EC + kt
                        nc.tensor.matmul(
                            pse[:],
                            saeb[:, k * BS:(k + 1) * BS],
                            ew[:, kt * D:(kt + 1) * D],
                            start=(k == 0), stop=(k == KT - 1),
                            skip_group_check=True,
                        )
                        nc.tensor.matmul(
                            psq[:],
                            saebsq[:, k * BS:(k + 1) * BS],
                            esq[:, kt * D:(kt + 1) * D],
                            start=(k == 0), stop=(k == KT - 1),
                            skip_group_check=True,
                        )

                load_consts()

                # inter[b, d] = sum_emb^2 - sum_sq   (0.5 folded into w1)
                # (ACT Square: single-PSUM-input rule for elementwise ops)
                inter = fmw.tile([128, D], BF16, tag="inter", name="inter")
                for dc in range(DT):
                    dsl = slice(dc * 128, (dc + 1) * 128)
                    tmp = fmst.tile([128, 128], F32, tag="tmp", name="tmp")
                    nc.scalar.activation(
                        tmp[:], pse[:, dsl],
                        mybir.ActivationFunctionType.Square)
                    nc.vector.tensor_sub(inter[:, dsl], tmp[:], psq[:, dsl])

                # interT via PE transpose (4x 128x128)
                interT = fmw.tile([128, D], BF16, tag="interT", name="interT")
                for dc in range(DT):
                    ptp = fmtp.tile([128, 128], BF16, tag="ptp", name="ptp")
                    nc.tensor.transpose(
                        ptp[:], inter[:, dc * 128:(dc + 1) * 128], idt[:])
                    nc.vector.tensor_copy(
                        interT[:, dc * 128:(dc + 1) * 128], ptp[:])

                # h shard [b, d2] = relu(interT.T @ w1 + b1)
                hps = fmhp.tile([128, D], F32, tag="hps", name="hps")
                for kd in range(DT):
                    nc.tensor.matmul(
                        hps[:],
                        interT[:, kd * 128:(kd + 1) * 128],
                        w1[:, kd * D:(kd + 1) * D],
                        start=(kd == 0), stop=False,
                        skip_group_check=True,
                    )
                nc.tensor.matmul(
                    hps[:], ones[:, :], b1r[:, :],
                    start=False, stop=True, skip_group_check=True,
                )
                hbf = fmst.tile([128, D], BF16, tag="hbf", name="hbf")
                nc.scalar.activation(
                    hbf[:], hps[:], mybir.ActivationFunctionType.Relu)

                # ship h shard to DRAM for the AllGather
                nc.gpsimd.dma_start(h_mine[:, :], hbf[:])

            if "fm" not in phases:
                load_consts()

            if "ag" in phases:
                nc.gpsimd.collective_compute(
                    "AllGather",
                    mybir.AluOpType.bypass,
                    replica_groups=[list(range(C))],
                    ins=[h_mine.opt()],
                    outs=[h_all.opt()],
                )
                for kd in range(DT):
                    nc.scalar.dma_start_transpose(
                        htall[:, kd * B:(kd + 1) * B],
                        h_all[:, kd * 128:(kd + 1) * 128])
            else:
                nc.vector.memset(htall[:], 0.01)

            # ---------------- Phase p2: linear GEMM (O shard) + mlp2 ------
            with tc.tile_pool(name=f"p2ps{rep}", bufs=1, space="PSUM") as p2ps:
                psm = [p2ps.tile([128, OS], F32, tag=f"psm{m}", name=f"psm{m}")
                       for m in range(MT)]
                if "p2" in phases:
                    for nb in range(NB):
                        sa = p2sa.tile([128, NTB * B], BF16, tag="sa", name="sa")
                        nc.sync.dma_start(
                            sa[:], saeTd[:, nb * NTB * B:(nb + 1) * NTB * B])
                        lw = p2lw.tile([128, NTB * OS], BF16, tag="lw", name="lw")
                        nc.scalar.dma_start(
                            lw[:], linwd[:, nb * NTB * OS:(nb + 1) * NTB * OS])
                        for m in range(MT):
                            for nt in range(NTB):
                                nc.tensor.matmul(
                                    psm[m][:],
                                    sa[:, nt * B + m * 128:nt * B + (m + 1) * 128],
                                    lw[:, nt * OS:(nt + 1) * OS],
                                    start=(nb == 0 and nt == 0),
                                    stop=False,
                                    skip_group_check=True,
                                )
                else:
                    for m in range(MT):
                        nc.tensor.matmul(
                            psm[m][:], ones[:, :], br[:, :],
                            start=True, stop=False, skip_group_check=True,
                        )

                # tail: mlp2 accumulated into the same PSUM banks + bias
                for m in range(MT):
                    if "tail" in phases:
                        for kd in range(DT):
                            nc.tensor.matmul(
                                psm[m][:],
                                htall[:, kd * B + m * 128:kd * B + (m + 1) * 128],
                                w2[:, kd * OS:(kd + 1) * OS],
                                start=False, stop=False,
                                skip_group_check=True,
                            )
                    nc.tensor.matmul(
                        psm[m][:], ones[:, :], br[:, :],
                        start=False, stop=True, skip_group_check=True,
                    )
                    ot = p2st.tile([128, OS], F32, tag="ot", name="ot")
                    nc.vector.tensor_copy(ot[:], psm[m][:])
                    nc.sync.dma_start(out[m * 128:(m + 1) * 128, :], ot[:])

    nc.compile()
    return nc


_CACHE = {}


def _get_nc():
    if "nc" not in _CACHE:
        _CACHE["nc"] = _build()
    return _CACHE["nc"]


def _swz(a, inner):
    """[K*128, inner] row-major -> [128, K*inner] partition-major flat."""
    k = a.shape[0] // 128
    return np.ascontiguousarray(
        a.reshape(k, 128, inner).transpose(1, 0, 2).reshape(128, k * inner))


def make_in_maps(sae_features, emb, linear_w, linear_b, mlp1_w, mlp1_b,
                 mlp2_w, mlp2_b):
    bf = ml_dtypes.bfloat16
    f32 = np.float32
    sae = np.asarray(sae_features, dtype=f32)
    emb_f = np.asarray(emb, dtype=f32)

    saeT = np.ascontiguousarray(sae.T)              # (N, B) f32
    saeT_bf = saeT.astype(bf)
    saeTd = _swz(saeT_bf, B)
    embd = _swz(emb_f.astype(bf), D)
    mlp1wT = np.ascontiguousarray((0.5 * np.asarray(mlp1_w, f32)).T).astype(bf)
    mlp1brow = np.asarray(mlp1_b, f32).reshape(1, D).astype(bf)
    mlp2wT_f = np.ascontiguousarray(np.asarray(mlp2_w, f32).T)  # (D, O)
    linwT_f = np.ascontiguousarray(np.asarray(linear_w, f32).T)  # (N, O)
    bias_f = np.asarray(linear_b, f32) + np.asarray(mlp2_b, f32)  # (O,)
    ident = np.eye(128, dtype=bf)

    in_maps = []
    for c in range(C):
        osl = slice(c * OS, (c + 1) * OS)
        bsl = slice(c * BS, (c + 1) * BS)
        sb = np.ascontiguousarray(saeT[:, bsl])
        in_maps.append({
            "saeTd": saeTd,
            "saebd": _swz(sb.astype(bf), BS),
            "embd": embd,
            "linwd": _swz(
                np.ascontiguousarray(linwT_f[:, osl]).astype(bf), OS),
            "mlp1wT": mlp1wT,
            "mlp1brow": mlp1brow,
            "mlp2wT": np.ascontiguousarray(mlp2wT_f[:, osl]).astype(bf),
            "biasrow": bias_f[osl].reshape(1, OS).astype(bf),
            "ident": ident,
        })
    return in_maps


def kernel(sae_features, emb, linear_w, linear_b, mlp1_w, mlp1_b, mlp2_w,
           mlp2_b):
    nc = _get_nc()
    in_maps = make_in_maps(
        sae_features, emb, linear_w, linear_b, mlp1_w, mlp1_b, mlp2_w, mlp2_b
    )
    res = run_bass_kernel_spmd(nc, in_maps, list(range(C)))
    full = np.empty((B, O), dtype=np.float32)
    for c in range(C):
        full[:, c * OS:(c + 1) * OS] = res.results[c]["out"]
    return full


# revision 4
# speedup vs baseline: 1.0601x; 1.0601x over previous
"""NeuralFactorizationMachine Trainium2 kernel v2 (8 NeuronCores, SPMD).

Reference computation (B=1024, N=16384, D=512, O=4096):
    sum_emb = sae @ emb                      (B, D)
    sum_sq  = (sae*sae) @ (emb*emb)          (B, D)
    inter   = 0.5*(sum_emb^2 - sum_sq)       (B, D)
    h       = relu(inter @ mlp1_w.T + b1)    (B, D)
    out     = h @ mlp2_w.T + b2 + sae @ linear_w.T + lb   (B, O)

Sharding (8 cores), v2 = batch-sharded FM (no big AllReduce):
  - FM GEMMs (sum_emb/sum_sq) are DATA-parallel over batch: core c owns
    batch rows c*128..(c+1)*128 and contracts over the FULL N=16384 with
    the full emb table (streamed).  This kills v1's 4 MiB fp32 AllReduce
    (~200 us of critical path) entirely.
  - mlp1 is batch-sharded too (h shard [128b, D] per core), then a tiny
    AllGather (128 KiB bf16 per rank) replicates h on every core; the
    gathered [B, D] block is loaded back via hardware DMA-transpose to
    give the [d, b] lhsT tiles mlp2 needs.
  - linear / mlp2 are tensor-parallel over O (each core owns 512 output
    cols); mlp2's 4 matmuls are accumulated INTO the linear GEMM's PSUM
    banks before stop, so the tail is just bias + copy + store.
  - Final output is a host-side concat over O.

All large inputs are host-swizzled to [128, ...] partition-major flat
layouts so every big DMA moves contiguous multi-KiB runs per partition.
DMA queue assignment: sync = saeb + sa stream + out; scalar = consts +
emb even chunks + lw stream + h transpose-loads; gpsimd(SWDGE) = saebsq
+ emb odd chunks + h_mine store + AllGather trigger.  GEMM inputs bf16,
fp32 PSUM accumulation.
"""

import numpy as np
import ml_dtypes

import concourse.bass as bass
import concourse.mybir as mybir
import concourse.tile as tile
from concourse import bacc
from concourse.bass_utils import run_bass_kernel_spmd

B, N, D, O = 1024, 16384, 512, 4096
C = 8                # cores
BS = B // C          # 128 batch rows per core (FM batch shard)
OS = O // C          # 512 output cols per core
BF16 = mybir.dt.bfloat16
F32 = mybir.dt.float32

KT = N // 128        # 128 n-ktiles total
EC = 4               # emb ktiles per streamed chunk
NCH = KT // EC       # 32 emb chunks
DT = D // 128        # 4 d-tiles
MT = B // 128        # 8 m-tiles (batch) for p2
NB = 16              # n-blocks for the linear GEMM stream
NTB = KT // NB       # 8 n-tiles per block
SBP = 8              # saeb load split


def _build(repeat=1, phases=("fm", "ag", "p2", "tail")):
    nc = bacc.Bacc(
        "TRN2",
        target_bir_lowering=False,
        debug=False,
        enable_asserts=False,
        num_devices=C,
    )

    # host-swizzled flat [128, ...] layouts (k-tile-major within each row)
    saeTd = nc.dram_tensor("saeTd", [128, KT * B], BF16, kind="ExternalInput").ap()
    saebd = nc.dram_tensor("saebd", [128, KT * BS], BF16, kind="ExternalInput").ap()
    embd = nc.dram_tensor("embd", [128, KT * D], BF16, kind="ExternalInput").ap()
    linwd = nc.dram_tensor("linwd", [128, KT * OS], BF16, kind="ExternalInput").ap()
    mlp1wT = nc.dram_tensor("mlp1wT", [D, D], BF16, kind="ExternalInput").ap()
    mlp1brow = nc.dram_tensor("mlp1brow", [1, D], BF16, kind="ExternalInput").ap()
    mlp2wT = nc.dram_tensor("mlp2wT", [D, OS], BF16, kind="ExternalInput").ap()
    biasrow = nc.dram_tensor("biasrow", [1, OS], BF16, kind="ExternalInput").ap()
    ident = nc.dram_tensor("ident", [128, 128], BF16, kind="ExternalInput").ap()
    out = nc.dram_tensor("out", [B, OS], F32, kind="ExternalOutput").ap()

    with tile.TileContext(nc) as tc:
      for rep in range(repeat):
        with (
            tc.tile_pool(name=f"dram{rep}", bufs=1, space="DRAM") as dram,
            tc.tile_pool(name=f"const{rep}", bufs=1) as cst,
            tc.tile_pool(name=f"p2sa{rep}", bufs=3) as p2sa,
            tc.tile_pool(name=f"p2lw{rep}", bufs=3) as p2lw,
            tc.tile_pool(name=f"p2st{rep}", bufs=4) as p2st,
            tc.tile_pool(name=f"fme{rep}", bufs=4) as fme,
        ):
            # prefetch the first emb chunks ahead of the consts so the FM
            # matmuls can start as early as possible
            ew_pre = []
            if "fm" in phases:
                for ch in range(2):
                    csl = slice(ch * EC * D, (ch + 1) * EC * D)
                    ew = fme.tile([128, EC * D], BF16, tag="ew", name="ew")
                    eng = nc.scalar if ch % 2 == 0 else nc.gpsimd
                    eng.dma_start(ew[:], embd[:, csl])
                    ew_pre.append(ew)

            # const tiles; DMAs are emitted after the FM stream so the
            # scalar queue serves the emb chunks first (consts are only
            # needed from ~60us on)
            w1 = cst.tile([128, DT * D], BF16, tag="w1", name="w1")
            b1r = cst.tile([1, D], BF16, tag="b1r", name="b1r")
            w2 = cst.tile([128, DT * OS], BF16, tag="w2", name="w2")
            br = cst.tile([1, OS], BF16, tag="br", name="br")
            idt = cst.tile([128, 128], BF16, tag="idt", name="idt")
            ones = cst.tile([1, 128], BF16, tag="ones", name="ones")
            nc.vector.memset(ones[:], 1.0)

            def load_consts():
                nc.scalar.dma_start(
                    w1[:], mlp1wT.rearrange("(k p) d -> p k d", p=128))
                nc.scalar.dma_start(b1r[:], mlp1brow[:, :])
                nc.scalar.dma_start(
                    w2[:], mlp2wT.rearrange("(k p) o -> p k o", p=128))
                nc.scalar.dma_start(br[:], biasrow[:, :])
                nc.scalar.dma_start(idt[:], ident[:, :])

            # h gathered from all cores: [B, D] bf16 (AllGather output)
            h_all = dram.tile([C * BS, D], BF16, tag="h_all",
                              name=f"h_all{rep}", addr_space="Shared")
            h_mine = dram.tile([BS, D], BF16, tag="h_mine",
                               name=f"h_mine{rep}")

            # hT tiles for mlp2 (filled by DMA-transpose from h_all):
            # htall[p, kd*B + b] = h_all[b, kd*128 + p]
            htall = cst.tile([128, DT * B], BF16, tag="htall", name="htall")

            # ---------------- Phase FM: batch-sharded FM GEMMs ------------
            if "fm" in phases:
              with (
                tc.tile_pool(name=f"fmw{rep}", bufs=1) as fmw,
                tc.tile_pool(name=f"fmps{rep}", bufs=1, space="PSUM") as fmps,
                tc.tile_pool(name=f"fmtp{rep}", bufs=2, space="PSUM") as fmtp,
                tc.tile_pool(name=f"fmhp{rep}", bufs=1, space="PSUM") as fmhp,
                tc.tile_pool(name=f"fmst{rep}", bufs=2) as fmst,
              ):
                # stationary operands: core's saeT column block, split into
                # SBP sub-DMAs so the first matmuls start early; squares
                # computed on DVE per sub-part (saves 4.2 MiB of DMA in the
                # critical early window).
                saeb = fmw.tile([128, KT * BS], BF16, tag="saeb", name="saeb")
                saebsq = fmw.tile([128, KT * BS], BF16, tag="saebsq",
                                  name="saebsq")
                part = KT * BS // SBP
                for s in range(SBP):
                    sl = slice(s * part, (s + 1) * part)
                    nc.sync.dma_start(saeb[:, sl], saebd[:, sl])
                    nc.vector.tensor_mul(saebsq[:, sl], saeb[:, sl],
                                         saeb[:, sl])

                pse = fmps.tile([128, D], F32, tag="pse", name="pse")
                psq = fmps.tile([128, D], F32, tag="psq", name="psq")

                for ch in range(NCH):
                    csl = slice(ch * EC * D, (ch + 1) * EC * D)
                    if ch < len(ew_pre):
                        ew = ew_pre[ch]
                    else:
                        ew = fme.tile([128, EC * D], BF16, tag="ew", name="ew")
                        # alternate queues so the emb stream isn't
                        # serialized behind a single DMA ring
                        eng = nc.scalar if ch % 2 == 0 else nc.gpsimd
                        eng.dma_start(ew[:], embd[:, csl])
                    esq = fme.tile([128, EC * D], BF16, tag="esq", name="esq")
                    nc.vector.tensor_mul(esq[:], ew[:], ew[:])
                    for kt in range(EC):
                        k = ch * EC + kt
                        nc.tensor.matmul(
                            pse[:],
                            saeb[:, k * BS:(k + 1) * BS],
                            ew[:, kt * D:(kt + 1) * D],
                            start=(k == 0), stop=(k == KT - 1),
                            skip_group_check=True,
                        )
                        nc.tensor.matmul(
                            psq[:],
                            saebsq[:, k * BS:(k + 1) * BS],
                            esq[:, kt * D:(kt + 1) * D],
                            start=(k == 0), stop=(k == KT - 1),
                            skip_group_check=True,
                        )

                load_consts()

                # inter[b, d] = sum_emb^2 - sum_sq   (0.5 folded into w1)
                # (ACT Square: single-PSUM-input rule for elementwise ops)
                inter = fmw.tile([128, D], BF16, tag="inter", name="inter")
                for dc in range(DT):
                    dsl = slice(dc * 128, (dc + 1) * 128)
                    tmp = fmst.tile([128, 128], F32, tag="tmp", name="tmp")
                    nc.scalar.activation(
                        tmp[:], pse[:, dsl],
                        mybir.ActivationFunctionType.Square)
                    nc.vector.tensor_sub(inter[:, dsl], tmp[:], psq[:, dsl])

                # interT via PE transpose (4x 128x128)
                interT = fmw.tile([128, D], BF16, tag="interT", name="interT")
                for dc in range(DT):
                    ptp = fmtp.tile([128, 128], BF16, tag="ptp", name="ptp")
                    nc.tensor.transpose(
                        ptp[:], inter[:, dc * 128:(dc + 1) * 128], idt[:])
                    nc.vector.tensor_copy(
                        interT[:, dc * 128:(dc + 1) * 128], ptp[:])

                # h shard [b, d2] = relu(interT.T @ w1 + b1)
                hps = fmhp.tile([128, D], F32, tag="hps", name="hps")
                for kd in range(DT):
                    nc.tensor.matmul(
                        hps[:],
                        interT[:, kd * 128:(kd + 1) * 128],
                        w1[:, kd * D:(kd + 1) * D],
                        start=(kd == 0), stop=False,
                        skip_group_check=True,
                    )
                nc.tensor.matmul(
                    hps[:], ones[:, :], b1r[:, :],
                    start=False, stop=True, skip_group_check=True,
                )
                hbf = fmst.tile([128, D], BF16, tag="hbf", name="hbf")
                nc.scalar.activation(
                    hbf[:], hps[:], mybir.ActivationFunctionType.Relu)

                # ship h shard to DRAM for the AllGather
                nc.gpsimd.dma_start(h_mine[:, :], hbf[:])

            if "fm" not in phases:
                load_consts()

            if "ag" in phases:
                nc.gpsimd.collective_compute(
                    "AllGather",
                    mybir.AluOpType.bypass,
                    replica_groups=[list(range(C))],
                    ins=[h_mine.opt()],
                    outs=[h_all.opt()],
                )
                for kd in range(DT):
                    nc.scalar.dma_start_transpose(
                        htall[:, kd * B:(kd + 1) * B],
                        h_all[:, kd * 128:(kd + 1) * 128])
            else:
                nc.vector.memset(htall[:], 0.01)

            # ---------------- Phase p2: linear GEMM (O shard) + mlp2 ------
            with tc.tile_pool(name=f"p2ps{rep}", bufs=1, space="PSUM") as p2ps:
                psm = [p2ps.tile([128, OS], F32, tag=f"psm{m}", name=f"psm{m}")
                       for m in range(MT)]
                if "p2" in phases:
                    for nb in range(NB):
                        sa = p2sa.tile([128, NTB * B], BF16, tag="sa", name="sa")
                        nc.sync.dma_start(
                            sa[:], saeTd[:, nb * NTB * B:(nb + 1) * NTB * B])
                        lw = p2lw.tile([128, NTB * OS], BF16, tag="lw", name="lw")
                        nc.scalar.dma_start(
                            lw[:], linwd[:, nb * NTB * OS:(nb + 1) * NTB * OS])
                        for m in range(MT):
                            for nt in range(NTB):
                                nc.tensor.matmul(
                                    psm[m][:],
                                    sa[:, nt * B + m * 128:nt * B + (m + 1) * 128],
                                    lw[:, nt * OS:(nt + 1) * OS],
                                    start=(nb == 0 and nt == 0),
                                    stop=False,
                                    skip_group_check=True,
                                )
                else:
                    for m in range(MT):
                        nc.tensor.matmul(
                            psm[m][:], ones[:, :], br[:, :],
                            start=True, stop=False, skip_group_check=True,
                        )

                # tail: mlp2 accumulated into the same PSUM banks + bias
                for m in range(MT):
                    if "tail" in phases:
                        for kd in range(DT):
                            nc.tensor.matmul(
                                psm[m][:],
                                htall[:, kd * B + m * 128:kd * B + (m + 1) * 128],
                                w2[:, kd * OS:(kd + 1) * OS],
                                start=False, stop=False,
                                skip_group_check=True,
                            )
                    nc.tensor.matmul(
                        psm[m][:], ones[:, :], br[:, :],
                        start=False, stop=True, skip_group_check=True,
                    )
                    ot = p2st.tile([128, OS], F32, tag="ot", name="ot")
                    nc.vector.tensor_copy(ot[:], psm[m][:])
                    nc.sync.dma_start(out[m * 128:(m + 1) * 128, :], ot[:])

    nc.compile()
    return nc


_CACHE = {}


def _get_nc():
    if "nc" not in _CACHE:
        _CACHE["nc"] = _build()
    return _CACHE["nc"]


def _swz(a, inner):
    """[K*128, inner] row-major -> [128, K*inner] partition-major flat."""
    k = a.shape[0] // 128
    return np.ascontiguousarray(
        a.reshape(k, 128, inner).transpose(1, 0, 2).reshape(128, k * inner))


def make_in_maps(sae_features, emb, linear_w, linear_b, mlp1_w, mlp1_b,
                 mlp2_w, mlp2_b):
    bf = ml_dtypes.bfloat16
    f32 = np.float32
    sae = np.asarray(sae_features, dtype=f32)
    emb_f = np.asarray(emb, dtype=f32)

    saeT = np.ascontiguousarray(sae.T)              # (N, B) f32
    saeT_bf = saeT.astype(bf)
    saeTd = _swz(saeT_bf, B)
    embd = _swz(emb_f.astype(bf), D)
    mlp1wT = np.ascontiguousarray((0.5 * np.asarray(mlp1_w, f32)).T).astype(bf)
    mlp1brow = np.asarray(mlp1_b, f32).reshape(1, D).astype(bf)
    mlp2wT_f = np.ascontiguousarray(np.asarray(mlp2_w, f32).T)  # (D, O)
    linwT_f = np.ascontiguousarray(np.asarray(linear_w, f32).T)  # (N, O)
    bias_f = np.asarray(linear_b, f32) + np.asarray(mlp2_b, f32)  # (O,)
    ident = np.eye(128, dtype=bf)

    in_maps = []
    for c in range(C):
        osl = slice(c * OS, (c + 1) * OS)
        bsl = slice(c * BS, (c + 1) * BS)
        sb = np.ascontiguousarray(saeT[:, bsl])
        in_maps.append({
            "saeTd": saeTd,
            "saebd": _swz(sb.astype(bf), BS),
            "embd": embd,
            "linwd": _swz(
                np.ascontiguousarray(linwT_f[:, osl]).astype(bf), OS),
            "mlp1wT": mlp1wT,
            "mlp1brow": mlp1brow,
            "mlp2wT": np.ascontiguousarray(mlp2wT_f[:, osl]).astype(bf),
            "biasrow": bias_f[osl].reshape(1, OS).astype(bf),
            "ident": ident,
        })
    return in_maps


def kernel(sae_features, emb, linear_w, linear_b, mlp1_w, mlp1_b, mlp2_w,
           mlp2_b):
    nc = _get_nc()
    in_maps = make_in_maps(
        sae_features, emb, linear_w, linear_b, mlp1_w, mlp1_b, mlp2_w, mlp2_b
    )
    res = run_bass_kernel_spmd(nc, in_maps, list(range(C)))
    full = np.empty((B, O), dtype=np.float32)
    for c in range(C):
        full[:, c * OS:(c + 1) * OS] = res.results[c]["out"]
    return full


# revision 6
# speedup vs baseline: 1.1305x; 1.0664x over previous
"""NeuralFactorizationMachine Trainium2 kernel v2 (8 NeuronCores, SPMD).

Reference computation (B=1024, N=16384, D=512, O=4096):
    sum_emb = sae @ emb                      (B, D)
    sum_sq  = (sae*sae) @ (emb*emb)          (B, D)
    inter   = 0.5*(sum_emb^2 - sum_sq)       (B, D)
    h       = relu(inter @ mlp1_w.T + b1)    (B, D)
    out     = h @ mlp2_w.T + b2 + sae @ linear_w.T + lb   (B, O)

Sharding (8 cores), v2 = batch-sharded FM (no big AllReduce):
  - FM GEMMs (sum_emb/sum_sq) are DATA-parallel over batch: core c owns
    batch rows c*128..(c+1)*128 and contracts over the FULL N=16384 with
    the full emb table (streamed).  This kills v1's 4 MiB fp32 AllReduce
    (~200 us of critical path) entirely.
  - mlp1 is batch-sharded too (h shard [128b, D] per core), then a tiny
    AllGather (128 KiB bf16 per rank) replicates h on every core; the
    gathered [B, D] block is loaded back via hardware DMA-transpose to
    give the [d, b] lhsT tiles mlp2 needs.
  - linear / mlp2 are tensor-parallel over O (each core owns 512 output
    cols); mlp2's 4 matmuls are accumulated INTO the linear GEMM's PSUM
    banks before stop, so the tail is just bias + copy + store.
  - Final output is a host-side concat over O.

All large inputs are host-swizzled to [128, ...] partition-major flat
layouts so every big DMA moves contiguous multi-KiB runs per partition.
DMA queue assignment: sync = saeb + sa stream + out; scalar = consts +
emb even chunks + lw stream + h transpose-loads; gpsimd(SWDGE) = saebsq
+ emb odd chunks + h_mine store + AllGather trigger.  GEMM inputs bf16,
fp32 PSUM accumulation.
"""

import numpy as np
import ml_dtypes

import concourse.bass as bass
import concourse.mybir as mybir
import concourse.tile as tile
from concourse import bacc
from concourse.bass_utils import run_bass_kernel_spmd

B, N, D, O = 1024, 16384, 512, 4096
C = 8                # cores
BS = B // C          # 128 batch rows per core (FM batch shard)
OS = O // C          # 512 output cols per core
BF16 = mybir.dt.bfloat16
F32 = mybir.dt.float32

KT = N // 128        # 128 n-ktiles total
EC = 4               # emb ktiles per streamed chunk
NCH = KT // EC       # 32 emb chunks
DT = D // 128        # 4 d-tiles
MT = B // 128        # 8 m-tiles (batch) for p2
NB = 16              # n-blocks for the linear GEMM stream
NTB = KT // NB       # 8 n-tiles per block
SBP = 16             # saeb load split


def _build(repeat=1, phases=("fm", "ag", "p2", "tail")):
    nc = bacc.Bacc(
        "TRN2",
        target_bir_lowering=False,
        debug=False,
        enable_asserts=False,
        num_devices=C,
    )

    # host-swizzled flat [128, ...] layouts (k-tile-major within each row)
    saeTd = nc.dram_tensor("saeTd", [128, KT * B], BF16, kind="ExternalInput").ap()
    saebd = nc.dram_tensor("saebd", [128, KT * BS], BF16, kind="ExternalInput").ap()
    embd = nc.dram_tensor("embd", [128, KT * D], BF16, kind="ExternalInput").ap()
    linwd = nc.dram_tensor("linwd", [128, KT * OS], BF16, kind="ExternalInput").ap()
    mlp1wT = nc.dram_tensor("mlp1wT", [D, D], BF16, kind="ExternalInput").ap()
    mlp1brow = nc.dram_tensor("mlp1brow", [1, D], BF16, kind="ExternalInput").ap()
    mlp2wT = nc.dram_tensor("mlp2wT", [D, OS], BF16, kind="ExternalInput").ap()
    biasrow = nc.dram_tensor("biasrow", [1, OS], BF16, kind="ExternalInput").ap()
    ident = nc.dram_tensor("ident", [128, 128], BF16, kind="ExternalInput").ap()
    out = nc.dram_tensor("out", [B, OS], F32, kind="ExternalOutput").ap()

    with tile.TileContext(nc) as tc:
      for rep in range(repeat):
        with (
            tc.tile_pool(name=f"dram{rep}", bufs=1, space="DRAM") as dram,
            tc.tile_pool(name=f"const{rep}", bufs=1) as cst,
            tc.tile_pool(name=f"p2sa{rep}", bufs=3) as p2sa,
            tc.tile_pool(name=f"p2lw{rep}", bufs=3) as p2lw,
            tc.tile_pool(name=f"p2st{rep}", bufs=4) as p2st,
            tc.tile_pool(name=f"fme{rep}", bufs=4) as fme,
        ):
            # prefetch the first emb chunks ahead of the consts so the FM
            # matmuls can start as early as possible
            ew_pre = []
            if "fm" in phases:
                for ch in range(4):
                    csl = slice(ch * EC * D, (ch + 1) * EC * D)
                    ew = fme.tile([128, EC * D], BF16, tag="ew", name="ew")
                    eng = nc.scalar if ch % 2 == 0 else nc.gpsimd
                    eng.dma_start(ew[:], embd[:, csl])
                    ew_pre.append(ew)

            # const tiles; DMAs are emitted after the FM stream so the
            # scalar queue serves the emb chunks first (consts are only
            # needed from ~60us on)
            w1 = cst.tile([128, DT * D], BF16, tag="w1", name="w1")
            b1r = cst.tile([1, D], BF16, tag="b1r", name="b1r")
            w2 = cst.tile([128, DT * OS], BF16, tag="w2", name="w2")
            br = cst.tile([1, OS], BF16, tag="br", name="br")
            idt = cst.tile([128, 128], BF16, tag="idt", name="idt")
            ones = cst.tile([1, 128], BF16, tag="ones", name="ones")
            nc.vector.memset(ones[:], 1.0)

            def load_consts():
                nc.scalar.dma_start(
                    w1[:], mlp1wT.rearrange("(k p) d -> p k d", p=128))
                nc.scalar.dma_start(b1r[:], mlp1brow[:, :])
                nc.scalar.dma_start(
                    w2[:], mlp2wT.rearrange("(k p) o -> p k o", p=128))
                nc.scalar.dma_start(br[:], biasrow[:, :])
                nc.scalar.dma_start(idt[:], ident[:, :])

            # h gathered from all cores: [B, D] bf16 (AllGather output)
            h_all = dram.tile([C * BS, D], BF16, tag="h_all",
                              name=f"h_all{rep}", addr_space="Shared")
            h_mine = dram.tile([BS, D], BF16, tag="h_mine",
                               name=f"h_mine{rep}")

            # hT tiles for mlp2 (filled by DMA-transpose from h_all):
            # htall[p, kd*B + b] = h_all[b, kd*128 + p]
            htall = cst.tile([128, DT * B], BF16, tag="htall", name="htall")

            # ---------------- Phase FM: batch-sharded FM GEMMs ------------
            if "fm" in phases:
              with (
                tc.tile_pool(name=f"fmw{rep}", bufs=1) as fmw,
                tc.tile_pool(name=f"fmps{rep}", bufs=1, space="PSUM") as fmps,
                tc.tile_pool(name=f"fmtp{rep}", bufs=2, space="PSUM") as fmtp,
                tc.tile_pool(name=f"fmhp{rep}", bufs=1, space="PSUM") as fmhp,
                tc.tile_pool(name=f"fmst{rep}", bufs=2) as fmst,
              ):
                # stationary operands: core's saeT column block, split into
                # SBP sub-DMAs so the first matmuls start early; squares
                # computed on DVE per sub-part (saves 4.2 MiB of DMA in the
                # critical early window).
                saeb = fmw.tile([128, KT * BS], BF16, tag="saeb", name="saeb")
                saebsq = fmw.tile([128, KT * BS], BF16, tag="saebsq",
                                  name="saebsq")
                part = KT * BS // SBP
                for s in range(SBP):
                    sl = slice(s * part, (s + 1) * part)
                    nc.sync.dma_start(saeb[:, sl], saebd[:, sl])

                pse = fmps.tile([128, D], F32, tag="pse", name="pse")
                psq = fmps.tile([128, D], F32, tag="psq", name="psq")

                chunks_per_part = NCH // SBP
                for ch in range(NCH):
                    # square the saeb part feeding this chunk group just-in-
                    # time so the DVE FIFO never head-blocks the esq squares
                    if ch % chunks_per_part == 0:
                        s = ch // chunks_per_part
                        sl = slice(s * part, (s + 1) * part)
                        nc.vector.tensor_mul(saebsq[:, sl], saeb[:, sl],
                                             saeb[:, sl])
                    csl = slice(ch * EC * D, (ch + 1) * EC * D)
                    if ch < len(ew_pre):
                        ew = ew_pre[ch]
                    else:
                        ew = fme.tile([128, EC * D], BF16, tag="ew", name="ew")
                        # alternate queues so the emb stream isn't
                        # serialized behind a single DMA ring
                        eng = nc.scalar if ch % 2 == 0 else nc.gpsimd
                        eng.dma_start(ew[:], embd[:, csl])
                    esq = fme.tile([128, EC * D], BF16, tag="esq", name="esq")
                    nc.vector.tensor_mul(esq[:], ew[:], ew[:])
                    for kt in range(EC):
                        k = ch * EC + kt
                        nc.tensor.matmul(
                            pse[:],
                            saeb[:, k * BS:(k + 1) * BS],
                            ew[:, kt * D:(kt + 1) * D],
                            start=(k == 0), stop=(k == KT - 1),
                            skip_group_check=True,
                        )
                        nc.tensor.matmul(
                            psq[:],
                            saebsq[:, k * BS:(k + 1) * BS],
                            esq[:, kt * D:(kt + 1) * D],
                            start=(k == 0), stop=(k == KT - 1),
                            skip_group_check=True,
                        )

                load_consts()

                # inter[b, d] = sum_emb^2 - sum_sq   (0.5 folded into w1)
                # (ACT Square: single-PSUM-input rule for elementwise ops)
                inter = fmw.tile([128, D], BF16, tag="inter", name="inter")
                for dc in range(DT):
                    dsl = slice(dc * 128, (dc + 1) * 128)
                    tmp = fmst.tile([128, 128], F32, tag="tmp", name="tmp")
                    nc.scalar.activation(
                        tmp[:], pse[:, dsl],
                        mybir.ActivationFunctionType.Square)
                    nc.vector.tensor_sub(inter[:, dsl], tmp[:], psq[:, dsl])

                # interT via PE transpose (4x 128x128)
                interT = fmw.tile([128, D], BF16, tag="interT", name="interT")
                for dc in range(DT):
                    ptp = fmtp.tile([128, 128], BF16, tag="ptp", name="ptp")
                    nc.tensor.transpose(
                        ptp[:], inter[:, dc * 128:(dc + 1) * 128], idt[:])
                    nc.vector.tensor_copy(
                        interT[:, dc * 128:(dc + 1) * 128], ptp[:])

                # h shard [b, d2] = relu(interT.T @ w1 + b1)
                hps = fmhp.tile([128, D], F32, tag="hps", name="hps")
                for kd in range(DT):
                    nc.tensor.matmul(
                        hps[:],
                        interT[:, kd * 128:(kd + 1) * 128],
                        w1[:, kd * D:(kd + 1) * D],
                        start=(kd == 0), stop=False,
                        skip_group_check=True,
                    )
                nc.tensor.matmul(
                    hps[:], ones[:, :], b1r[:, :],
                    start=False, stop=True, skip_group_check=True,
                )
                hbf = fmst.tile([128, D], BF16, tag="hbf", name="hbf")
                nc.scalar.activation(
                    hbf[:], hps[:], mybir.ActivationFunctionType.Relu)

                # ship h shard to DRAM for the AllGather
                nc.gpsimd.dma_start(h_mine[:, :], hbf[:])

            if "fm" not in phases:
                load_consts()

            if "ag" in phases:
                nc.gpsimd.collective_compute(
                    "AllGather",
                    mybir.AluOpType.bypass,
                    replica_groups=[list(range(C))],
                    ins=[h_mine.opt()],
                    outs=[h_all.opt()],
                )
                for kd in range(DT):
                    nc.scalar.dma_start_transpose(
                        htall[:, kd * B:(kd + 1) * B],
                        h_all[:, kd * 128:(kd + 1) * 128])
            else:
                nc.vector.memset(htall[:], 0.01)

            # ---------------- Phase p2: linear GEMM (O shard) + mlp2 ------
            with tc.tile_pool(name=f"p2ps{rep}", bufs=1, space="PSUM") as p2ps:
                psm = [p2ps.tile([128, OS], F32, tag=f"psm{m}", name=f"psm{m}")
                       for m in range(MT)]
                if "p2" in phases:
                    for nb in range(NB):
                        sa = p2sa.tile([128, NTB * B], BF16, tag="sa", name="sa")
                        nc.sync.dma_start(
                            sa[:], saeTd[:, nb * NTB * B:(nb + 1) * NTB * B])
                        lw = p2lw.tile([128, NTB * OS], BF16, tag="lw", name="lw")
                        nc.scalar.dma_start(
                            lw[:], linwd[:, nb * NTB * OS:(nb + 1) * NTB * OS])
                        for m in range(MT):
                            for nt in range(NTB):
                                nc.tensor.matmul(
                                    psm[m][:],
                                    sa[:, nt * B + m * 128:nt * B + (m + 1) * 128],
                                    lw[:, nt * OS:(nt + 1) * OS],
                                    start=(nb == 0 and nt == 0),
                                    stop=False,
                                    skip_group_check=True,
                                )
                else:
                    for m in range(MT):
                        nc.tensor.matmul(
                            psm[m][:], ones[:, :], br[:, :],
                            start=True, stop=False, skip_group_check=True,
                        )

                # tail: mlp2 accumulated into the same PSUM banks + bias
                for m in range(MT):
                    if "tail" in phases:
                        for kd in range(DT):
                            nc.tensor.matmul(
                                psm[m][:],
                                htall[:, kd * B + m * 128:kd * B + (m + 1) * 128],
                                w2[:, kd * OS:(kd + 1) * OS],
                                start=False, stop=False,
                                skip_group_check=True,
                            )
                    nc.tensor.matmul(
                        psm[m][:], ones[:, :], br[:, :],
                        start=False, stop=True, skip_group_check=True,
                    )
                    ot = p2st.tile([128, OS], F32, tag="ot", name="ot")
                    nc.vector.tensor_copy(ot[:], psm[m][:])
                    nc.sync.dma_start(out[m * 128:(m + 1) * 128, :], ot[:])

    nc.compile()
    return nc


_CACHE = {}


def _get_nc():
    if "nc" not in _CACHE:
        _CACHE["nc"] = _build()
    return _CACHE["nc"]


def _swz(a, inner):
    """[K*128, inner] row-major -> [128, K*inner] partition-major flat."""
    k = a.shape[0] // 128
    return np.ascontiguousarray(
        a.reshape(k, 128, inner).transpose(1, 0, 2).reshape(128, k * inner))


def make_in_maps(sae_features, emb, linear_w, linear_b, mlp1_w, mlp1_b,
                 mlp2_w, mlp2_b):
    bf = ml_dtypes.bfloat16
    f32 = np.float32
    sae = np.asarray(sae_features, dtype=f32)
    emb_f = np.asarray(emb, dtype=f32)

    saeT = np.ascontiguousarray(sae.T)              # (N, B) f32
    saeT_bf = saeT.astype(bf)
    saeTd = _swz(saeT_bf, B)
    embd = _swz(emb_f.astype(bf), D)
    mlp1wT = np.ascontiguousarray((0.5 * np.asarray(mlp1_w, f32)).T).astype(bf)
    mlp1brow = np.asarray(mlp1_b, f32).reshape(1, D).astype(bf)
    mlp2wT_f = np.ascontiguousarray(np.asarray(mlp2_w, f32).T)  # (D, O)
    linwT_f = np.ascontiguousarray(np.asarray(linear_w, f32).T)  # (N, O)
    bias_f = np.asarray(linear_b, f32) + np.asarray(mlp2_b, f32)  # (O,)
    ident = np.eye(128, dtype=bf)

    in_maps = []
    for c in range(C):
        osl = slice(c * OS, (c + 1) * OS)
        bsl = slice(c * BS, (c + 1) * BS)
        sb = np.ascontiguousarray(saeT[:, bsl])
        in_maps.append({
            "saeTd": saeTd,
            "saebd": _swz(sb.astype(bf), BS),
            "embd": embd,
            "linwd": _swz(
                np.ascontiguousarray(linwT_f[:, osl]).astype(bf), OS),
            "mlp1wT": mlp1wT,
            "mlp1brow": mlp1brow,
            "mlp2wT": np.ascontiguousarray(mlp2wT_f[:, osl]).astype(bf),
            "biasrow": bias_f[osl].reshape(1, OS).astype(bf),
            "ident": ident,
        })
    return in_maps


def kernel(sae_features, emb, linear_w, linear_b, mlp1_w, mlp1_b, mlp2_w,
           mlp2_b):
    nc = _get_nc()
    in_maps = make_in_maps(
        sae_features, emb, linear_w, linear_b, mlp1_w, mlp1_b, mlp2_w, mlp2_b
    )
    res = run_bass_kernel_spmd(nc, in_maps, list(range(C)))
    full = np.empty((B, O), dtype=np.float32)
    for c in range(C):
        full[:, c * OS:(c + 1) * OS] = res.results[c]["out"]
    return full


# revision 7
# speedup vs baseline: 1.1399x; 1.0084x over previous
"""NeuralFactorizationMachine Trainium2 kernel v2 (8 NeuronCores, SPMD).

Reference computation (B=1024, N=16384, D=512, O=4096):
    sum_emb = sae @ emb                      (B, D)
    sum_sq  = (sae*sae) @ (emb*emb)          (B, D)
    inter   = 0.5*(sum_emb^2 - sum_sq)       (B, D)
    h       = relu(inter @ mlp1_w.T + b1)    (B, D)
    out     = h @ mlp2_w.T + b2 + sae @ linear_w.T + lb   (B, O)

Sharding (8 cores), v2 = batch-sharded FM (no big AllReduce):
  - FM GEMMs (sum_emb/sum_sq) are DATA-parallel over batch: core c owns
    batch rows c*128..(c+1)*128 and contracts over the FULL N=16384 with
    the full emb table (streamed).  This kills v1's 4 MiB fp32 AllReduce
    (~200 us of critical path) entirely.
  - mlp1 is batch-sharded too (h shard [128b, D] per core), then a tiny
    AllGather (128 KiB bf16 per rank) replicates h on every core; the
    gathered [B, D] block is loaded back via hardware DMA-transpose to
    give the [d, b] lhsT tiles mlp2 needs.
  - linear / mlp2 are tensor-parallel over O (each core owns 512 output
    cols); mlp2's 4 matmuls are accumulated INTO the linear GEMM's PSUM
    banks before stop, so the tail is just bias + copy + store.
  - Final output is a host-side concat over O.

All large inputs are host-swizzled to [128, ...] partition-major flat
layouts so every big DMA moves contiguous multi-KiB runs per partition.
DMA queue assignment: sync = saeb + sa stream + out; scalar = consts +
emb even chunks + lw stream + h transpose-loads; gpsimd(SWDGE) = saebsq
+ emb odd chunks + h_mine store + AllGather trigger.  GEMM inputs bf16,
fp32 PSUM accumulation.
"""

import numpy as np
import ml_dtypes

import concourse.bass as bass
import concourse.mybir as mybir
import concourse.tile as tile
from concourse import bacc
from concourse.bass_utils import run_bass_kernel_spmd

B, N, D, O = 1024, 16384, 512, 4096
C = 8                # cores
BS = B // C          # 128 batch rows per core (FM batch shard)
OS = O // C          # 512 output cols per core
BF16 = mybir.dt.bfloat16
F32 = mybir.dt.float32

KT = N // 128        # 128 n-ktiles total
EC = 4               # emb ktiles per streamed chunk
NCH = KT // EC       # 32 emb chunks
DT = D // 128        # 4 d-tiles
MT = B // 128        # 8 m-tiles (batch) for p2
NB = 16              # n-blocks for the linear GEMM stream
NTB = KT // NB       # 8 n-tiles per block
SBP = 16             # saeb load split


def _build(repeat=1, phases=("fm", "ag", "p2", "tail")):
    nc = bacc.Bacc(
        "TRN2",
        target_bir_lowering=False,
        debug=False,
        enable_asserts=False,
        num_devices=C,
    )

    # host-swizzled flat [128, ...] layouts (k-tile-major within each row)
    saeTd = nc.dram_tensor("saeTd", [128, KT * B], BF16, kind="ExternalInput").ap()
    saebd = nc.dram_tensor("saebd", [128, KT * BS], BF16, kind="ExternalInput").ap()
    embd = nc.dram_tensor("embd", [128, KT * D], BF16, kind="ExternalInput").ap()
    linwd = nc.dram_tensor("linwd", [128, KT * OS], BF16, kind="ExternalInput").ap()
    mlp1wT = nc.dram_tensor("mlp1wT", [D, D], BF16, kind="ExternalInput").ap()
    mlp1brow = nc.dram_tensor("mlp1brow", [1, D], BF16, kind="ExternalInput").ap()
    mlp2wT = nc.dram_tensor("mlp2wT", [D, OS], BF16, kind="ExternalInput").ap()
    biasrow = nc.dram_tensor("biasrow", [1, OS], BF16, kind="ExternalInput").ap()
    ident = nc.dram_tensor("ident", [128, 128], BF16, kind="ExternalInput").ap()
    out = nc.dram_tensor("out", [B, OS], F32, kind="ExternalOutput").ap()

    with tile.TileContext(nc) as tc:
      for rep in range(repeat):
        with (
            tc.tile_pool(name=f"dram{rep}", bufs=1, space="DRAM") as dram,
            tc.tile_pool(name=f"const{rep}", bufs=1) as cst,
            tc.tile_pool(name=f"p2sa{rep}", bufs=3) as p2sa,
            tc.tile_pool(name=f"p2lw{rep}", bufs=3) as p2lw,
            tc.tile_pool(name=f"p2st{rep}", bufs=4) as p2st,
            tc.tile_pool(name=f"fme{rep}", bufs=4) as fme,
        ):
            # prefetch the first emb chunks ahead of the consts so the FM
            # matmuls can start as early as possible
            ew_pre = []
            if "fm" in phases:
                for ch in range(4):
                    csl = slice(ch * EC * D, (ch + 1) * EC * D)
                    ew = fme.tile([128, EC * D], BF16, tag="ew", name="ew")
                    eng = nc.scalar if ch % 2 == 0 else nc.gpsimd
                    eng.dma_start(ew[:], embd[:, csl])
                    ew_pre.append(ew)

            # const tiles; DMAs are emitted after the FM stream so the
            # scalar queue serves the emb chunks first (consts are only
            # needed from ~60us on)
            w1 = cst.tile([128, DT * D], BF16, tag="w1", name="w1")
            b1r = cst.tile([1, D], BF16, tag="b1r", name="b1r")
            w2 = cst.tile([128, DT * OS], BF16, tag="w2", name="w2")
            br = cst.tile([1, OS], BF16, tag="br", name="br")
            idt = cst.tile([128, 128], BF16, tag="idt", name="idt")
            ones = cst.tile([1, 128], BF16, tag="ones", name="ones")
            nc.vector.memset(ones[:], 1.0)

            def load_consts():
                nc.scalar.dma_start(
                    w1[:], mlp1wT.rearrange("(k p) d -> p k d", p=128))
                nc.scalar.dma_start(b1r[:], mlp1brow[:, :])
                nc.scalar.dma_start(
                    w2[:], mlp2wT.rearrange("(k p) o -> p k o", p=128))
                nc.scalar.dma_start(br[:], biasrow[:, :])
                nc.scalar.dma_start(idt[:], ident[:, :])

            # h gathered from all cores: [B, D] bf16 (AllGather output)
            h_all = dram.tile([C * BS, D], BF16, tag="h_all",
                              name=f"h_all{rep}", addr_space="Shared")
            h_mine = dram.tile([BS, D], BF16, tag="h_mine",
                               name=f"h_mine{rep}")

            # hT tiles for mlp2 (filled by DMA-transpose from h_all):
            # htall[p, kd*B + b] = h_all[b, kd*128 + p]
            htall = cst.tile([128, DT * B], BF16, tag="htall", name="htall")

            fm_mid_dma = [None]

            # ---------------- Phase FM: batch-sharded FM GEMMs ------------
            if "fm" in phases:
              with (
                tc.tile_pool(name=f"fmw{rep}", bufs=1) as fmw,
                tc.tile_pool(name=f"fmps{rep}", bufs=1, space="PSUM") as fmps,
                tc.tile_pool(name=f"fmtp{rep}", bufs=2, space="PSUM") as fmtp,
                tc.tile_pool(name=f"fmhp{rep}", bufs=1, space="PSUM") as fmhp,
                tc.tile_pool(name=f"fmst{rep}", bufs=2) as fmst,
              ):
                # stationary operands: core's saeT column block, split into
                # SBP sub-DMAs so the first matmuls start early; squares
                # computed on DVE per sub-part (saves 4.2 MiB of DMA in the
                # critical early window).
                saeb = fmw.tile([128, KT * BS], BF16, tag="saeb", name="saeb")
                saebsq = fmw.tile([128, KT * BS], BF16, tag="saebsq",
                                  name="saebsq")
                part = KT * BS // SBP
                for s in range(SBP):
                    sl = slice(s * part, (s + 1) * part)
                    nc.sync.dma_start(saeb[:, sl], saebd[:, sl])

                pse = fmps.tile([128, D], F32, tag="pse", name="pse")
                psq = fmps.tile([128, D], F32, tag="psq", name="psq")

                chunks_per_part = NCH // SBP
                ew_dmas = []
                for ch in range(NCH):
                    # square the saeb part feeding this chunk group just-in-
                    # time so the DVE FIFO never head-blocks the esq squares
                    if ch % chunks_per_part == 0:
                        s = ch // chunks_per_part
                        sl = slice(s * part, (s + 1) * part)
                        nc.vector.tensor_mul(saebsq[:, sl], saeb[:, sl],
                                             saeb[:, sl])
                    csl = slice(ch * EC * D, (ch + 1) * EC * D)
                    if ch < len(ew_pre):
                        ew = ew_pre[ch]
                    else:
                        ew = fme.tile([128, EC * D], BF16, tag="ew", name="ew")
                        # alternate queues so the emb stream isn't
                        # serialized behind a single DMA ring
                        eng = nc.scalar if ch % 2 == 0 else nc.gpsimd
                        ew_dmas.append(eng.dma_start(ew[:], embd[:, csl]))
                    esq = fme.tile([128, EC * D], BF16, tag="esq", name="esq")
                    nc.vector.tensor_mul(esq[:], ew[:], ew[:])
                    for kt in range(EC):
                        k = ch * EC + kt
                        nc.tensor.matmul(
                            pse[:],
                            saeb[:, k * BS:(k + 1) * BS],
                            ew[:, kt * D:(kt + 1) * D],
                            start=(k == 0), stop=(k == KT - 1),
                            skip_group_check=True,
                        )
                        nc.tensor.matmul(
                            psq[:],
                            saebsq[:, k * BS:(k + 1) * BS],
                            esq[:, kt * D:(kt + 1) * D],
                            start=(k == 0), stop=(k == KT - 1),
                            skip_group_check=True,
                        )

                load_consts()

                # inter[b, d] = sum_emb^2 - sum_sq   (0.5 folded into w1)
                # (ACT Square: single-PSUM-input rule for elementwise ops)
                inter = fmw.tile([128, D], BF16, tag="inter", name="inter")
                for dc in range(DT):
                    dsl = slice(dc * 128, (dc + 1) * 128)
                    tmp = fmst.tile([128, 128], F32, tag="tmp", name="tmp")
                    nc.scalar.activation(
                        tmp[:], pse[:, dsl],
                        mybir.ActivationFunctionType.Square)
                    nc.vector.tensor_sub(inter[:, dsl], tmp[:], psq[:, dsl])

                # interT via PE transpose (4x 128x128)
                interT = fmw.tile([128, D], BF16, tag="interT", name="interT")
                for dc in range(DT):
                    ptp = fmtp.tile([128, 128], BF16, tag="ptp", name="ptp")
                    nc.tensor.transpose(
                        ptp[:], inter[:, dc * 128:(dc + 1) * 128], idt[:])
                    nc.vector.tensor_copy(
                        interT[:, dc * 128:(dc + 1) * 128], ptp[:])

                # h shard [b, d2] = relu(interT.T @ w1 + b1)
                hps = fmhp.tile([128, D], F32, tag="hps", name="hps")
                for kd in range(DT):
                    nc.tensor.matmul(
                        hps[:],
                        interT[:, kd * 128:(kd + 1) * 128],
                        w1[:, kd * D:(kd + 1) * D],
                        start=(kd == 0), stop=False,
                        skip_group_check=True,
                    )
                nc.tensor.matmul(
                    hps[:], ones[:, :], b1r[:, :],
                    start=False, stop=True, skip_group_check=True,
                )
                hbf = fmst.tile([128, D], BF16, tag="hbf", name="hbf")
                nc.scalar.activation(
                    hbf[:], hps[:], mybir.ActivationFunctionType.Relu)

                # ship h shard to DRAM for the AllGather
                nc.gpsimd.dma_start(h_mine[:, :], hbf[:])
                fm_mid_dma[0] = ew_dmas[len(ew_dmas) // 2]

            if "fm" not in phases:
                load_consts()

            if "ag" in phases:
                nc.gpsimd.collective_compute(
                    "AllGather",
                    mybir.AluOpType.bypass,
                    replica_groups=[list(range(C))],
                    ins=[h_mine.opt()],
                    outs=[h_all.opt()],
                )
                for kd in range(DT):
                    nc.scalar.dma_start_transpose(
                        htall[:, kd * B:(kd + 1) * B],
                        h_all[:, kd * 128:(kd + 1) * 128])
            else:
                nc.vector.memset(htall[:], 0.01)

            # ---------------- Phase p2: linear GEMM (O shard) + mlp2 ------
            with tc.tile_pool(name=f"p2ps{rep}", bufs=1, space="PSUM") as p2ps:
                psm = [p2ps.tile([128, OS], F32, tag=f"psm{m}", name=f"psm{m}")
                       for m in range(MT)]
                if "p2" in phases:
                    from concourse.tile import add_dep_helper
                    for nb in range(NB):
                        sa = p2sa.tile([128, NTB * B], BF16, tag="sa", name="sa")
                        sad = nc.sync.dma_start(
                            sa[:], saeTd[:, nb * NTB * B:(nb + 1) * NTB * B])
                        lw = p2lw.tile([128, NTB * OS], BF16, tag="lw", name="lw")
                        lwd = nc.scalar.dma_start(
                            lw[:], linwd[:, nb * NTB * OS:(nb + 1) * NTB * OS])
                        if nb < 2 and fm_mid_dma[0] is not None:
                            # hold the p2 prefetch off the HBM bus until the
                            # emb stream is half done (contention ordering,
                            # not a data dependency)
                            add_dep_helper(
                                sad.ins, fm_mid_dma[0].ins, sync=True,
                                reason="delay p2 sa prefetch behind emb")
                            add_dep_helper(
                                lwd.ins, fm_mid_dma[0].ins, sync=True,
                                reason="delay p2 lw prefetch behind emb")
                        for m in range(MT):
                            for nt in range(NTB):
                                nc.tensor.matmul(
                                    psm[m][:],
                                    sa[:, nt * B + m * 128:nt * B + (m + 1) * 128],
                                    lw[:, nt * OS:(nt + 1) * OS],
                                    start=(nb == 0 and nt == 0),
                                    stop=False,
                                    skip_group_check=True,
                                )
                else:
                    for m in range(MT):
                        nc.tensor.matmul(
                            psm[m][:], ones[:, :], br[:, :],
                            start=True, stop=False, skip_group_check=True,
                        )

                # tail: mlp2 accumulated into the same PSUM banks + bias
                for m in range(MT):
                    if "tail" in phases:
                        for kd in range(DT):
                            nc.tensor.matmul(
                                psm[m][:],
                                htall[:, kd * B + m * 128:kd * B + (m + 1) * 128],
                                w2[:, kd * OS:(kd + 1) * OS],
                                start=False, stop=False,
                                skip_group_check=True,
                            )
                    nc.tensor.matmul(
                        psm[m][:], ones[:, :], br[:, :],
                        start=False, stop=True, skip_group_check=True,
                    )
                    ot = p2st.tile([128, OS], F32, tag="ot", name="ot")
                    nc.vector.tensor_copy(ot[:], psm[m][:])
                    nc.sync.dma_start(out[m * 128:(m + 1) * 128, :], ot[:])

    nc.compile()
    return nc


_CACHE = {}


def _get_nc():
    if "nc" not in _CACHE:
        _CACHE["nc"] = _build()
    return _CACHE["nc"]


def _swz(a, inner):
    """[K*128, inner] row-major -> [128, K*inner] partition-major flat."""
    k = a.shape[0] // 128
    return np.ascontiguousarray(
        a.reshape(k, 128, inner).transpose(1, 0, 2).reshape(128, k * inner))


def make_in_maps(sae_features, emb, linear_w, linear_b, mlp1_w, mlp1_b,
                 mlp2_w, mlp2_b):
    bf = ml_dtypes.bfloat16
    f32 = np.float32
    sae = np.asarray(sae_features, dtype=f32)
    emb_f = np.asarray(emb, dtype=f32)

    saeT = np.ascontiguousarray(sae.T)              # (N, B) f32
    saeT_bf = saeT.astype(bf)
    saeTd = _swz(saeT_bf, B)
    embd = _swz(emb_f.astype(bf), D)
    mlp1wT = np.ascontiguousarray((0.5 * np.asarray(mlp1_w, f32)).T).astype(bf)
    mlp1brow = np.asarray(mlp1_b, f32).reshape(1, D).astype(bf)
    mlp2wT_f = np.ascontiguousarray(np.asarray(mlp2_w, f32).T)  # (D, O)
    linwT_f = np.ascontiguousarray(np.asarray(linear_w, f32).T)  # (N, O)
    bias_f = np.asarray(linear_b, f32) + np.asarray(mlp2_b, f32)  # (O,)
    ident = np.eye(128, dtype=bf)

    in_maps = []
    for c in range(C):
        osl = slice(c * OS, (c + 1) * OS)
        bsl = slice(c * BS, (c + 1) * BS)
        sb = np.ascontiguousarray(saeT[:, bsl])
        in_maps.append({
            "saeTd": saeTd,
            "saebd": _swz(sb.astype(bf), BS),
            "embd": embd,
            "linwd": _swz(
                np.ascontiguousarray(linwT_f[:, osl]).astype(bf), OS),
            "mlp1wT": mlp1wT,
            "mlp1brow": mlp1brow,
            "mlp2wT": np.ascontiguousarray(mlp2wT_f[:, osl]).astype(bf),
            "biasrow": bias_f[osl].reshape(1, OS).astype(bf),
            "ident": ident,
        })
    return in_maps


def kernel(sae_features, emb, linear_w, linear_b, mlp1_w, mlp1_b, mlp2_w,
           mlp2_b):
    nc = _get_nc()
    in_maps = make_in_maps(
        sae_features, emb, linear_w, linear_b, mlp1_w, mlp1_b, mlp2_w, mlp2_b
    )
    res = run_bass_kernel_spmd(nc, in_maps, list(range(C)))
    full = np.empty((B, O), dtype=np.float32)
    for c in range(C):
        full[:, c * OS:(c + 1) * OS] = res.results[c]["out"]
    return full


# revision 10
# speedup vs baseline: 1.1905x; 1.0444x over previous
"""NeuralFactorizationMachine Trainium2 kernel v2 (8 NeuronCores, SPMD).

Reference computation (B=1024, N=16384, D=512, O=4096):
    sum_emb = sae @ emb                      (B, D)
    sum_sq  = (sae*sae) @ (emb*emb)          (B, D)
    inter   = 0.5*(sum_emb^2 - sum_sq)       (B, D)
    h       = relu(inter @ mlp1_w.T + b1)    (B, D)
    out     = h @ mlp2_w.T + b2 + sae @ linear_w.T + lb   (B, O)

Sharding (8 cores), v2 = batch-sharded FM (no big AllReduce):
  - FM GEMMs (sum_emb/sum_sq) are DATA-parallel over batch: core c owns
    batch rows c*128..(c+1)*128 and contracts over the FULL N=16384 with
    the full emb table (streamed).  This kills v1's 4 MiB fp32 AllReduce
    (~200 us of critical path) entirely.
  - mlp1 is batch-sharded too (h shard [128b, D] per core), then a tiny
    AllGather (128 KiB bf16 per rank) replicates h on every core; the
    gathered [B, D] block is loaded back via hardware DMA-transpose to
    give the [d, b] lhsT tiles mlp2 needs.
  - linear / mlp2 are tensor-parallel over O (each core owns 512 output
    cols); mlp2's 4 matmuls are accumulated INTO the linear GEMM's PSUM
    banks before stop, so the tail is just bias + copy + store.
  - Final output is a host-side concat over O.

All large inputs are host-swizzled to [128, ...] partition-major flat
layouts so every big DMA moves contiguous multi-KiB runs per partition.
DMA queue assignment: sync = saeb + sa stream + out; scalar = consts +
emb even chunks + lw stream + h transpose-loads; gpsimd(SWDGE) = saebsq
+ emb odd chunks + h_mine store + AllGather trigger.  GEMM inputs bf16,
fp32 PSUM accumulation.
"""

import numpy as np
import ml_dtypes

import concourse.bass as bass
import concourse.mybir as mybir
import concourse.tile as tile
from concourse import bacc
from concourse.bass_utils import run_bass_kernel_spmd

B, N, D, O = 1024, 16384, 512, 4096
C = 8                # cores
BS = B // C          # 128 batch rows per core (FM batch shard)
OS = O // C          # 512 output cols per core
BF16 = mybir.dt.bfloat16
F32 = mybir.dt.float32

KT = N // 128        # 128 n-ktiles total
EC = 4               # emb ktiles per streamed chunk
NCH = KT // EC       # 32 emb chunks
DT = D // 128        # 4 d-tiles
MT = B // 128        # 8 m-tiles (batch) for p2
NB = 16              # n-blocks for the linear GEMM stream
NTB = KT // NB       # 8 n-tiles per block
SBP = 16             # saeb load split


def _build(repeat=1, phases=("fm", "ag", "p2", "tail")):
    nc = bacc.Bacc(
        "TRN2",
        target_bir_lowering=False,
        debug=False,
        enable_asserts=False,
        num_devices=C,
    )

    # host-swizzled flat [128, ...] layouts (k-tile-major within each row)
    saeTd = nc.dram_tensor("saeTd", [128, KT * B], BF16, kind="ExternalInput").ap()
    saebd = nc.dram_tensor("saebd", [128, KT * BS], BF16, kind="ExternalInput").ap()
    embd = nc.dram_tensor("embd", [128, KT * D], BF16, kind="ExternalInput").ap()
    linwd = nc.dram_tensor("linwd", [128, KT * OS], BF16, kind="ExternalInput").ap()
    mlp1wT = nc.dram_tensor("mlp1wT", [D, D], BF16, kind="ExternalInput").ap()
    mlp1brow = nc.dram_tensor("mlp1brow", [1, D], BF16, kind="ExternalInput").ap()
    mlp2wT = nc.dram_tensor("mlp2wT", [D, OS], BF16, kind="ExternalInput").ap()
    biasrow = nc.dram_tensor("biasrow", [1, OS], BF16, kind="ExternalInput").ap()
    ident = nc.dram_tensor("ident", [128, 128], BF16, kind="ExternalInput").ap()
    out = nc.dram_tensor("out", [B, OS], F32, kind="ExternalOutput").ap()

    with tile.TileContext(nc) as tc:
      for rep in range(repeat):
        with (
            tc.tile_pool(name=f"dram{rep}", bufs=1, space="DRAM") as dram,
            tc.tile_pool(name=f"const{rep}", bufs=1) as cst,
            tc.tile_pool(name=f"p2sa{rep}", bufs=3) as p2sa,
            tc.tile_pool(name=f"p2lw{rep}", bufs=3) as p2lw,
            tc.tile_pool(name=f"p2st{rep}", bufs=4) as p2st,
            tc.tile_pool(name=f"fme{rep}", bufs=4) as fme,
        ):
            # prefetch the first emb chunks ahead of the consts so the FM
            # matmuls can start as early as possible
            ew_pre = []
            if "fm" in phases:
                for ch in range(4):
                    csl = slice(ch * EC * D, (ch + 1) * EC * D)
                    ew = fme.tile([128, EC * D], BF16, tag="ew", name="ew")
                    eng = nc.scalar if ch % 2 == 0 else nc.gpsimd
                    eng.dma_start(ew[:], embd[:, csl])
                    ew_pre.append(ew)

            # const tiles; DMAs are emitted after the FM stream so the
            # scalar queue serves the emb chunks first (consts are only
            # needed from ~60us on)
            w1 = cst.tile([128, DT * D], BF16, tag="w1", name="w1")
            b1r = cst.tile([1, D], BF16, tag="b1r", name="b1r")
            w2 = cst.tile([128, DT * OS], BF16, tag="w2", name="w2")
            br = cst.tile([1, OS], BF16, tag="br", name="br")
            idt = cst.tile([128, 128], BF16, tag="idt", name="idt")
            ones = cst.tile([1, 128], BF16, tag="ones", name="ones")
            nc.vector.memset(ones[:], 1.0)

            def load_consts():
                nc.scalar.dma_start(
                    w1[:], mlp1wT.rearrange("(k p) d -> p k d", p=128))
                nc.scalar.dma_start(b1r[:], mlp1brow[:, :])
                nc.scalar.dma_start(
                    w2[:], mlp2wT.rearrange("(k p) o -> p k o", p=128))
                nc.scalar.dma_start(br[:], biasrow[:, :])
                nc.scalar.dma_start(idt[:], ident[:, :])

            # h gathered from all cores: [B, D] bf16 (AllGather output)
            h_all = dram.tile([C * BS, D], BF16, tag="h_all",
                              name=f"h_all{rep}", addr_space="Shared")
            h_mine = dram.tile([BS, D], BF16, tag="h_mine",
                               name=f"h_mine{rep}")

            # hT tiles for mlp2 (filled by DMA-transpose from h_all):
            # htall[p, kd*B + b] = h_all[b, kd*128 + p]
            htall = cst.tile([128, DT * B], BF16, tag="htall", name="htall")

            fm_mid_dma = [None]

            # ---------------- Phase FM: batch-sharded FM GEMMs ------------
            if "fm" in phases:
              with (
                tc.tile_pool(name=f"fmw{rep}", bufs=1) as fmw,
                tc.tile_pool(name=f"fmps{rep}", bufs=1, space="PSUM") as fmps,
                tc.tile_pool(name=f"fmtp{rep}", bufs=2, space="PSUM") as fmtp,
                tc.tile_pool(name=f"fmhp{rep}", bufs=1, space="PSUM") as fmhp,
                tc.tile_pool(name=f"fmst{rep}", bufs=2) as fmst,
              ):
                # stationary operands: core's saeT column block, split into
                # SBP sub-DMAs so the first matmuls start early; squares
                # computed on DVE per sub-part (saves 4.2 MiB of DMA in the
                # critical early window).
                saeb = fmw.tile([128, KT * BS], BF16, tag="saeb", name="saeb")
                saebsq = fmw.tile([128, KT * BS], BF16, tag="saebsq",
                                  name="saebsq")
                part = KT * BS // SBP
                for s in range(SBP):
                    sl = slice(s * part, (s + 1) * part)
                    nc.sync.dma_start(saeb[:, sl], saebd[:, sl])

                pse = fmps.tile([128, D], F32, tag="pse", name="pse")
                psq = fmps.tile([128, D], F32, tag="psq", name="psq")

                chunks_per_part = NCH // SBP
                ew_dmas = []
                for ch in range(NCH):
                    # square the saeb part feeding this chunk group just-in-
                    # time so the DVE FIFO never head-blocks the esq squares
                    if ch % chunks_per_part == 0:
                        s = ch // chunks_per_part
                        sl = slice(s * part, (s + 1) * part)
                        nc.vector.tensor_mul(saebsq[:, sl], saeb[:, sl],
                                             saeb[:, sl])
                    csl = slice(ch * EC * D, (ch + 1) * EC * D)
                    if ch < len(ew_pre):
                        ew = ew_pre[ch]
                    else:
                        ew = fme.tile([128, EC * D], BF16, tag="ew", name="ew")
                        # alternate queues so the emb stream isn't
                        # serialized behind a single DMA ring
                        eng = nc.scalar if ch % 2 == 0 else nc.gpsimd
                        ew_dmas.append(eng.dma_start(ew[:], embd[:, csl]))
                    esq = fme.tile([128, EC * D], BF16, tag="esq", name="esq")
                    nc.vector.tensor_mul(esq[:], ew[:], ew[:])
                    for kt in range(EC):
                        k = ch * EC + kt
                        nc.tensor.matmul(
                            pse[:],
                            saeb[:, k * BS:(k + 1) * BS],
                            ew[:, kt * D:(kt + 1) * D],
                            start=(k == 0), stop=(k == KT - 1),
                            skip_group_check=True,
                        )
                        nc.tensor.matmul(
                            psq[:],
                            saebsq[:, k * BS:(k + 1) * BS],
                            esq[:, kt * D:(kt + 1) * D],
                            start=(k == 0), stop=(k == KT - 1),
                            skip_group_check=True,
                        )

                load_consts()

                # inter[b, d] = sum_emb^2 - sum_sq   (0.5 folded into w1)
                # (ACT Square: single-PSUM-input rule for elementwise ops)
                inter = fmw.tile([128, D], BF16, tag="inter", name="inter")
                for dc in range(DT):
                    dsl = slice(dc * 128, (dc + 1) * 128)
                    tmp = fmst.tile([128, 128], F32, tag="tmp", name="tmp")
                    nc.scalar.activation(
                        tmp[:], pse[:, dsl],
                        mybir.ActivationFunctionType.Square)
                    nc.vector.tensor_sub(inter[:, dsl], tmp[:], psq[:, dsl])

                # interT via PE transpose (4x 128x128)
                interT = fmw.tile([128, D], BF16, tag="interT", name="interT")
                for dc in range(DT):
                    ptp = fmtp.tile([128, 128], BF16, tag="ptp", name="ptp")
                    nc.tensor.transpose(
                        ptp[:], inter[:, dc * 128:(dc + 1) * 128], idt[:])
                    nc.vector.tensor_copy(
                        interT[:, dc * 128:(dc + 1) * 128], ptp[:])

                # h shard [b, d2] = relu(interT.T @ w1 + b1)
                hps = fmhp.tile([128, D], F32, tag="hps", name="hps")
                for kd in range(DT):
                    nc.tensor.matmul(
                        hps[:],
                        interT[:, kd * 128:(kd + 1) * 128],
                        w1[:, kd * D:(kd + 1) * D],
                        start=(kd == 0), stop=False,
                        skip_group_check=True,
                    )
                nc.tensor.matmul(
                    hps[:], ones[:, :], b1r[:, :],
                    start=False, stop=True, skip_group_check=True,
                )
                hbf = fmst.tile([128, D], BF16, tag="hbf", name="hbf")
                nc.scalar.activation(
                    hbf[:], hps[:], mybir.ActivationFunctionType.Relu)

                # ship h shard to DRAM for the AllGather
                nc.gpsimd.dma_start(h_mine[:, :], hbf[:])
                fm_mid_dma[0] = ew_dmas[len(ew_dmas) // 2]

            if "fm" not in phases:
                load_consts()

            if "ag" in phases:
                nc.gpsimd.collective_compute(
                    "AllGather",
                    mybir.AluOpType.bypass,
                    replica_groups=[list(range(C))],
                    ins=[h_mine.opt()],
                    outs=[h_all.opt()],
                )
                for kd in range(DT):
                    nc.scalar.dma_start_transpose(
                        htall[:, kd * B:(kd + 1) * B],
                        h_all[:, kd * 128:(kd + 1) * 128])
            else:
                nc.vector.memset(htall[:], 0.01)

            # ---------------- Phase p2: linear GEMM (O shard) + mlp2 ------
            # per-m PSUM pools so each bank releases right after its tail
            # copy -- the next repeat's FM matmuls can then grab banks while
            # this repeat's tail is still draining (shrinks the inter-rep
            # PE gap below the HAM re-throttle window).
            if True:
                # opened in reverse so m=0's pool sits at the TOP of the
                # allocator stack: the forward tail loop then pops pools in
                # legal LIFO order, freeing banks progressively.
                p2ps_cms = [None] * MT
                p2ps_pools = [None] * MT
                for m in reversed(range(MT)):
                    p2ps_cms[m] = tc.tile_pool(
                        name=f"p2ps{rep}_{m}", bufs=1, space="PSUM")
                    p2ps_pools[m] = p2ps_cms[m].__enter__()
                psm = [p2ps_pools[m].tile([128, OS], F32, tag=f"psm{m}",
                                          name=f"psm{m}")
                       for m in range(MT)]
                if "p2" in phases:
                    from concourse.tile import add_dep_helper
                    for nb in range(NB):
                        sa = p2sa.tile([128, NTB * B], BF16, tag="sa", name="sa")
                        sad = nc.sync.dma_start(
                            sa[:], saeTd[:, nb * NTB * B:(nb + 1) * NTB * B])
                        lw = p2lw.tile([128, NTB * OS], BF16, tag="lw", name="lw")
                        lwd = nc.scalar.dma_start(
                            lw[:], linwd[:, nb * NTB * OS:(nb + 1) * NTB * OS])
                        if nb < 2 and fm_mid_dma[0] is not None:
                            # hold the p2 prefetch off the HBM bus until the
                            # emb stream is half done (contention ordering,
                            # not a data dependency)
                            add_dep_helper(
                                sad.ins, fm_mid_dma[0].ins, sync=True,
                                reason="delay p2 sa prefetch behind emb")
                            add_dep_helper(
                                lwd.ins, fm_mid_dma[0].ins, sync=True,
                                reason="delay p2 lw prefetch behind emb")
                        for m in range(MT):
                            for nt in range(NTB):
                                nc.tensor.matmul(
                                    psm[m][:],
                                    sa[:, nt * B + m * 128:nt * B + (m + 1) * 128],
                                    lw[:, nt * OS:(nt + 1) * OS],
                                    start=(nb == 0 and nt == 0),
                                    stop=False,
                                    skip_group_check=True,
                                )
                else:
                    for m in range(MT):
                        nc.tensor.matmul(
                            psm[m][:], ones[:, :], br[:, :],
                            start=True, stop=False, skip_group_check=True,
                        )

                # tail: mlp2 accumulated into the same PSUM banks + bias
                for m in range(MT):
                    if "tail" in phases:
                        for kd in range(DT):
                            nc.tensor.matmul(
                                psm[m][:],
                                htall[:, kd * B + m * 128:kd * B + (m + 1) * 128],
                                w2[:, kd * OS:(kd + 1) * OS],
                                start=False, stop=False,
                                skip_group_check=True,
                            )
                    nc.tensor.matmul(
                        psm[m][:], ones[:, :], br[:, :],
                        start=False, stop=True, skip_group_check=True,
                    )
                    ot = p2st.tile([128, OS], F32, tag="ot", name="ot")
                    nc.vector.tensor_copy(ot[:], psm[m][:])
                    nc.sync.dma_start(out[m * 128:(m + 1) * 128, :], ot[:])
                    p2ps_cms[m].__exit__(None, None, None)

    nc.compile()
    return nc


_CACHE = {}


def _get_nc():
    if "nc" not in _CACHE:
        _CACHE["nc"] = _build()
    return _CACHE["nc"]


def _swz(a, inner):
    """[K*128, inner] row-major -> [128, K*inner] partition-major flat."""
    k = a.shape[0] // 128
    return np.ascontiguousarray(
        a.reshape(k, 128, inner).transpose(1, 0, 2).reshape(128, k * inner))


def make_in_maps(sae_features, emb, linear_w, linear_b, mlp1_w, mlp1_b,
                 mlp2_w, mlp2_b):
    bf = ml_dtypes.bfloat16
    f32 = np.float32
    sae = np.asarray(sae_features, dtype=f32)
    emb_f = np.asarray(emb, dtype=f32)

    saeT = np.ascontiguousarray(sae.T)              # (N, B) f32
    saeT_bf = saeT.astype(bf)
    saeTd = _swz(saeT_bf, B)
    embd = _swz(emb_f.astype(bf), D)
    mlp1wT = np.ascontiguousarray((0.5 * np.asarray(mlp1_w, f32)).T).astype(bf)
    mlp1brow = np.asarray(mlp1_b, f32).reshape(1, D).astype(bf)
    mlp2wT_f = np.ascontiguousarray(np.asarray(mlp2_w, f32).T)  # (D, O)
    linwT_f = np.ascontiguousarray(np.asarray(linear_w, f32).T)  # (N, O)
    bias_f = np.asarray(linear_b, f32) + np.asarray(mlp2_b, f32)  # (O,)
    ident = np.eye(128, dtype=bf)

    in_maps = []
    for c in range(C):
        osl = slice(c * OS, (c + 1) * OS)
        bsl = slice(c * BS, (c + 1) * BS)
        sb = np.ascontiguousarray(saeT[:, bsl])
        in_maps.append({
            "saeTd": saeTd,
            "saebd": _swz(sb.astype(bf), BS),
            "embd": embd,
            "linwd": _swz(
                np.ascontiguousarray(linwT_f[:, osl]).astype(bf), OS),
            "mlp1wT": mlp1wT,
            "mlp1brow": mlp1brow,
            "mlp2wT": np.ascontiguousarray(mlp2wT_f[:, osl]).astype(bf),
            "biasrow": bias_f[osl].reshape(1, OS).astype(bf),
            "ident": ident,
        })
    return in_maps


def kernel(sae_features, emb, linear_w, linear_b, mlp1_w, mlp1_b, mlp2_w,
           mlp2_b):
    nc = _get_nc()
    in_maps = make_in_maps(
        sae_features, emb, linear_w, linear_b, mlp1_w, mlp1_b, mlp2_w, mlp2_b
    )
    res = run_bass_kernel_spmd(nc, in_maps, list(range(C)))
    full = np.empty((B, O), dtype=np.float32)
    for c in range(C):
        full[:, c * OS:(c + 1) * OS] = res.results[c]["out"]
    return full
